# revision 19
# baseline (speedup 1.0000x reference)
"""DeepPoly ReLU transformer back-substitution on 8 trn2 NeuronCores.

Math (reference, per output row n of weight W [N, M]):
    l, u = bounds;  ind2 = l>=0;  ind3 = (u>0)&(l<0)
    beta = 1[ind2];  lmbda = ind2?1 : ind3? u/(u-l) : 0;  mu = ind3? -l*u/(u-l) : 0
    new_l = max(diag(beta)W,0)@in_l + min(diag(beta)W,0)@in_u + beta*bias
    new_u = max(diag(lmbda)W,0)@in_u + min(diag(lmbda)W,0)@in_l + (mu+lmbda*bias)
    lb = max(ind2? l:0, new_l);  ub = min(ind2|ind3? u:0, new_u)

Since beta, lmbda >= 0 the pos/neg splits factor through the scalars.  With
Wp = max(W,0), Wn = min(W,0), s = in_l + in_u, d = in_l - in_u:
    a := Wp@in_l + Wn@in_u = (W@s + |W|@d) / 2
    b := Wp@in_u + Wn@in_l = (W@s - |W|@d) / 2
so the device only needs TWO matvecs, W@s and |W|@d, against a single fp8
stream of W.  The problem is memory-bound: fp8 halves the HBM traffic vs
bf16 (8.39 MB/core, ~23.4us at the 358 GB/s per-core HBM roofline), and the
DeepPoly clamp margins (~20 sigma) make the matvec precision irrelevant.

Per core (row-shard of N/8=1024 output rows, sharded by columns of W^T):
  - W^T streams HBM->SBUF as fp8e4 (host pre-scales by 64 to avoid the fp8
    subnormal range; results are divided back on host).
  - |W| is produced on the DVE with ONE bitwise-AND per tile: fp8e4 is
    sign-magnitude, so AND 0x7F7F7F7F on the int32-viewed tile computes the
    elementwise absolute value of 4 packed fp8 lanes per 32-bit op.
  - The PE runs FOUR concurrent column-group streams (tile_position cols
    0/32/64/96): W@s for output cols 0-511 and 512-1023, |W|@d likewise.
    Each group streams 64 accumulating [128,512] fp8 matmuls.
  - PSUM rows 0/32/64/96 accumulate over the M=8192 contraction, are
    DVE-copied to SBUF and DMA'd out raw.  All O(N)/O(M) prep and the O(N)
    epilogue (bias, beta/lmbda scaling, clamping) run on host.
"""

import numpy as np

import concourse.bass as bass
import concourse.mybir as mybir
from concourse.tile import TileContext
from concourse.bass_utils import run_bass_kernel_spmd

N = 8192          # output rows of W
M = 8192          # contraction dim (input features)
NC = 8            # cores
NPC = N // NC     # 1024 output rows per core
MT = M // 128     # 64 contraction subtiles of 128

F8 = mybir.dt.float8e4
U32 = mybir.dt.uint32
F32 = mybir.dt.float32

WSCALE = np.float32(64.0)  # host pre-scale of W into fp8e4 normal range

# DMA tile schedule, in 128-row m-subtiles per transfer.  1 MiB steady
# tiles keep the SDMA engines saturated (8 KB packets, per-transfer receipt
# stalls amortized and hidden by the other ring); small tiles at both ends
# cut the latency to the first matmul and the drain-out tail.  The phase-
# split matmul order hides the per-tile |W| AND latency.  Must sum to MT.
TILE_SCHED = [1, 1, 2, 4, 4] + [8] * 6 + [2, 1, 1]
assert sum(TILE_SCHED) == MT

N_WARM = 8  # cold-clock PE warmup matmuls issued during the DMA preamble

_nc_cache = {}


def _build():
    nc = bass.Bass()
    # host pre-tiles W^T so each [128, A*NPC] DMA tile is one contiguous
    # block: tile t partition p holds rows {m0_t + a*128 + p} of W^T[:, core]
    wt = nc.dram_tensor("wt", [M * NPC], F8, kind="ExternalInput")
    vecs = nc.dram_tensor("vecs", [128, 2 * MT], F8, kind="ExternalInput")
    # raw PSUM image; host reads rows 0/32/64/96
    outm = nc.dram_tensor("outm", [97, 512], F32, kind="ExternalOutput")

    with TileContext(nc) as tc:
        with (
            tc.tile_pool(name="wpool", bufs=7) as wpool,
            tc.tile_pool(name="wapool", bufs=7) as wapool,
            tc.tile_pool(name="const", bufs=1) as cpool,
            tc.tile_pool(name="psum", bufs=1, space="PSUM") as ppool,
            tc.tile_pool(name="epil", bufs=1) as epool,
        ):
            vecs_sb = cpool.tile([128, 2 * MT], F8, tag="vecs")
            nc.scalar.dma_start(out=vecs_sb[:], in_=vecs[:])

            # PE warmup: dep-free matmuls on memset scratch keep the PE busy
            # through the HAM SHORT window while the first W tile loads, so
            # real matmuls run at 2.4 GHz from the start.
            scratch = cpool.tile([128, 512], F8, tag="scratch")
            nc.gpsimd.memset(scratch[:], 0.0)
            warm_ps = ppool.tile([2, 512], F32, tag="warm", name="warm_ps")
            for _ in range(N_WARM):
                nc.tensor.matmul(
                    warm_ps[:],
                    scratch[:, 0:2],
                    scratch[:, 0:512],
                    start=True,
                    stop=True,
                )

            # accumulators: row 0 = W@s cols 0-511, row 32 = W@s cols
            # 512-1023, row 64 = |W|@d cols 0-511, row 96 = |W|@d cols
            # 512-1023 (output partition == PE column-group offset)
            ps = ppool.tile([97, 512], F32, tag="ps", name="ps")

            mt = 0
            ofs = 0
            # greedy byte-balance across the two HWDGE rings (SP / ACT) so
            # neither ring lags: a lagging ring stalls the in-order PE queue
            # on that ring's tiles even while the other ring's data sits in
            # SBUF.  scalar starts with the vecs transfer queued.
            ring_bytes = {0: 0.0, 1: 0.125}
            for t, A in enumerate(TILE_SCHED):
                w = wpool.tile([128, A * NPC], F8, tag="w", name="w")
                r = 0 if ring_bytes[0] <= ring_bytes[1] else 1
                ring_bytes[r] += A
                dma_eng = nc.sync if r == 0 else nc.scalar
                dma_eng.dma_start(
                    out=w[:],
                    in_=wt[ofs : ofs + 128 * A * NPC].rearrange(
                        "(p f) -> p f", p=128
                    ),
                )
                ofs += 128 * A * NPC
                # |W| tile: fp8e4 is sign-magnitude, so clearing the top bit
                # of every byte is elementwise abs; one u32 AND handles 4
                # fp8 lanes -> 2 elem/cycle/partition on the DVE.
                wa = wapool.tile([128, A * NPC // 4], U32, tag="wa", name="wa")
                nc.vector.tensor_scalar(
                    out=wa[:],
                    in0=w[:].bitcast(U32),
                    scalar1=0x7F7F7F7F,
                    scalar2=None,
                    op0=mybir.AluOpType.bitwise_and,
                )
                # emit all W matmuls of the tile first (they only depend on
                # the DMA), then all |W| matmuls (which wait on the AND): the
                # in-order PE queue then never stalls real W work behind the
                # DVE, and adjacent tiles' W/|W| phases overlap so all four
                # col-group streams stay busy.
                for a in range(A):
                    ma = mt + a
                    sv = vecs_sb[:, 2 * ma : 2 * ma + 1]
                    lo = a * NPC
                    st = ma == 0
                    sp = ma == MT - 1
                    nc.tensor.matmul(
                        ps[0:1, :],
                        sv,
                        w[:, lo : lo + 512],
                        start=st,
                        stop=sp,
                        tile_position=(0, 0),
                        skip_group_check=True,
                    )
                    nc.tensor.matmul(
                        ps[32:33, :],
                        sv,
                        w[:, lo + 512 : lo + 1024],
                        start=st,
                        stop=sp,
                        tile_position=(0, 32),
                        skip_group_check=True,
                    )
                for a in range(A):
                    ma = mt + a
                    dv = vecs_sb[:, 2 * ma + 1 : 2 * ma + 2]
                    q = a * NPC // 4
                    st = ma == 0
                    sp = ma == MT - 1
                    nc.tensor.matmul(
                        ps[64:65, :],
                        dv,
                        wa[:, q : q + 128].bitcast(F8),
                        start=st,
                        stop=sp,
                        tile_position=(0, 64),
                        skip_group_check=True,
                    )
                    nc.tensor.matmul(
                        ps[96:97, :],
                        dv,
                        wa[:, q + 128 : q + 256].bitcast(F8),
                        start=st,
                        stop=sp,
                        tile_position=(0, 96),
                        skip_group_check=True,
                    )
                mt += A
                # dep-free filler matmuls at tile boundaries keep the PE HAM
                # activity window non-idle across supply gaps.  The early ramp
                # (while the first big tiles stream in) gets bigger fillers so
                # the PE clock reaches 2.4 GHz before the steady state.
                if t < len(TILE_SCHED) - 1:
                    fcols = 256 if t < 5 else 64
                    for _ in range(3 if t < 5 else 2):
                        nc.tensor.matmul(
                            warm_ps[:, 0:fcols],
                            scratch[:, 0:2],
                            scratch[:, 0:fcols],
                            start=True,
                            stop=True,
                        )

            # evacuate PSUM through SBUF and out to HBM.  Four single-row
            # transfers: a multi-partition SBUF->DRAM transfer serializes all
            # its per-partition descriptors onto ONE SDMA engine (~22 GB/s),
            # so a [97,512] store costs ~9 us while 4 tiny rows cost ~1.5 us.
            om = epool.tile([97, 512], F32, tag="om")
            nc.vector.tensor_copy(om[:], ps[:])
            nc.sync.dma_start(out=outm[0:1, :], in_=om[0:1, :])
            nc.scalar.dma_start(out=outm[32:33, :], in_=om[32:33, :])
            nc.sync.dma_start(out=outm[64:65, :], in_=om[64:65, :])
            nc.scalar.dma_start(out=outm[96:97, :], in_=om[96:97, :])
    return nc


def _legalize_sync_waits(nc):
    """The walrus codegen in this toolchain accepts at most ONE sync-wait per
    instruction ("Too many sync wait commands").  Tile freely attaches
    several.  Hoist all but the last wait of each offending instruction onto
    same-engine NOPs spliced immediately before it — same-queue waits execute
    in order, so semantics are identical."""
    nop_map = {}
    all_nops = set()
    for f in nc.m.functions:
        for b in f.blocks:
            for inst in list(b.instructions):
                si = inst.sync_info
                if not (si and si.on_wait and len(si.on_wait) > 1):
                    continue
                waits = list(si.on_wait)
                nops = []
                for w in waits[:-1]:
                    # engine.nop() appends to the current (last) bb; the
                    # splice below removes it from wherever it landed and
                    # re-inserts it right before its target instruction.
                    nop = nc.engines[inst.engine].nop()
                    nop.ins.sync_info = mybir.SyncInfo(on_wait=[w], on_update=[])
                    nops.append(nop.ins)
                    all_nops.add(nop.ins.name)
                inst.sync_info = mybir.SyncInfo(
                    on_wait=[waits[-1]], on_update=list(si.on_update or [])
                )
                nop_map[inst.name] = nops
    if not nop_map:
        return
    for f in nc.m.functions:
        for b in f.blocks:
            insts = b.instructions
            new_list = []
            for inst in insts:
                if inst.name in all_nops:
                    continue
                for nop in nop_map.get(inst.name, ()):
                    new_list.append(nop)
                new_list.append(inst)
            insts[:] = new_list


def get_nc():
    if "fp8" not in _nc_cache:
        nc = _build()
        _legalize_sync_waits(nc)
        _nc_cache["fp8"] = nc
    return _nc_cache["fp8"]


def host_prep(bounds, weight, bias, in_lower, in_upper):
    f8 = np.dtype(mybir.dt.np(F8))
    f32 = np.float32
    weight = np.asarray(weight, f32)
    in_lower = np.asarray(in_lower, f32)
    in_upper = np.asarray(in_upper, f32)

    s = (in_lower + in_upper).astype(f32)
    d = (in_lower - in_upper).astype(f32)
    # per m-subtile stationary columns: [s, d]
    mvecs = np.stack([s, d], axis=1).astype(f8)
    vecs = np.ascontiguousarray(
        mvecs.reshape(MT, 128, 2).transpose(1, 0, 2).reshape(128, 2 * MT)
    )

    WT = np.ascontiguousarray((weight.T * WSCALE).astype(f8))  # [M, N]
    in_maps = []
    for c in range(NC):
        sl = slice(c * NPC, (c + 1) * NPC)
        Wc = WT[:, sl]
        blocks = []
        m0 = 0
        for A in TILE_SCHED:
            blocks.append(
                Wc[m0 : m0 + A * 128]
                .reshape(A, 128, NPC)
                .transpose(1, 0, 2)
                .reshape(-1)
            )
            m0 += A * 128
        wt_flat = np.ascontiguousarray(np.concatenate(blocks))
        in_maps.append({"wt": wt_flat, "vecs": vecs})
    return in_maps


def assemble(results, bounds, bias):
    """Host epilogue: combine the raw matvecs with the O(N) DeepPoly
    coefficient math, exactly mirroring the reference formulas in fp32."""
    f32 = np.float32
    bounds = np.asarray(bounds, f32)
    bias = np.asarray(bias, f32)
    l, u = bounds[0], bounds[1]
    ind2 = l >= 0
    ind3 = (u > 0) & (l < 0)
    one, zero = f32(1.0), f32(0.0)
    diff = np.where(ind3, u - l, one).astype(f32)
    lmbda = np.where(ind2, one, np.where(ind3, u / diff, zero)).astype(f32)
    beta = np.where(ind2, one, zero).astype(f32)
    mu = np.where(ind3, -l * u / diff, zero).astype(f32)
    lb0 = np.where(ind2, l, zero).astype(f32)
    ub0 = np.where(ind2, u, np.where(ind3, u, zero)).astype(f32)

    a = np.empty(N, f32)
    b = np.empty(N, f32)
    inv = f32(1.0) / (f32(2.0) * WSCALE)
    for c, r in enumerate(results):
        sl = slice(c * NPC, (c + 1) * NPC)
        om = np.asarray(r["outm"], f32)  # raw [97, 512] PSUM image
        ws = np.concatenate([om[0], om[32]])   # W@s, scaled by WSCALE
        ad = np.concatenate([om[64], om[96]])  # |W|@d, scaled by WSCALE
        a[sl] = (ws + ad) * inv
        b[sl] = (ws - ad) * inv

    new_l = (beta * (a + bias)).astype(f32)
    new_u = (lmbda * (b + bias) + mu).astype(f32)
    lb = np.maximum(lb0, new_l)
    ub = np.minimum(ub0, new_u)
    return np.stack([lb, ub]).astype(f32)


def kernel(bounds, weight, bias, in_lower, in_upper):
    nc = get_nc()
    in_maps = host_prep(bounds, weight, bias, in_lower, in_upper)
    res = run_bass_kernel_spmd(nc, in_maps, list(range(NC)))
    return assemble(res.results, bounds, bias)


# revision 24
# speedup vs baseline: 1.0852x; 1.0852x over previous
"""DeepPoly ReLU transformer back-substitution on 8 trn2 NeuronCores.

Math (reference, per output row n of weight W [N, M]):
    l, u = bounds;  ind2 = l>=0;  ind3 = (u>0)&(l<0)
    beta = 1[ind2];  lmbda = ind2?1 : ind3? u/(u-l) : 0;  mu = ind3? -l*u/(u-l) : 0
    new_l = max(diag(beta)W,0)@in_l + min(diag(beta)W,0)@in_u + beta*bias
    new_u = max(diag(lmbda)W,0)@in_u + min(diag(lmbda)W,0)@in_l + (mu+lmbda*bias)
    lb = max(ind2? l:0, new_l);  ub = min(ind2|ind3? u:0, new_u)

Since beta, lmbda >= 0 the pos/neg splits factor through the scalars.  With
Wp = max(W,0), Wn = min(W,0), s = in_l + in_u, d = in_l - in_u:
    a := Wp@in_l + Wn@in_u = (W@s + |W|@d) / 2
    b := Wp@in_u + Wn@in_l = (W@s - |W|@d) / 2
so the device only needs TWO matvecs, W@s and |W|@d, against a single fp8
stream of W.  The problem is memory-bound: fp8 halves the HBM traffic vs
bf16 (8.39 MB/core, ~23.4us at the 358 GB/s per-core HBM roofline), and the
DeepPoly clamp margins (~20 sigma) make the matvec precision irrelevant.

Per core (row-shard of N/8=1024 output rows, sharded by columns of W^T):
  - W^T streams HBM->SBUF as fp8e4 (host pre-scales by 64 to avoid the fp8
    subnormal range; results are divided back on host).
  - |W| is produced on the DVE with ONE bitwise-AND per tile: fp8e4 is
    sign-magnitude, so AND 0x7F7F7F7F on the int32-viewed tile computes the
    elementwise absolute value of 4 packed fp8 lanes per 32-bit op.
  - The PE runs FOUR concurrent column-group streams (tile_position cols
    0/32/64/96): W@s for output cols 0-511 and 512-1023, |W|@d likewise.
    Each group streams 64 accumulating [128,512] fp8 matmuls.
  - PSUM rows 0/32/64/96 accumulate over the M=8192 contraction, are
    DVE-copied to SBUF and DMA'd out raw.  All O(N)/O(M) prep and the O(N)
    epilogue (bias, beta/lmbda scaling, clamping) run on host.
"""

import numpy as np

import concourse.bass as bass
import concourse.mybir as mybir
from concourse.tile import TileContext
from concourse.bass_utils import run_bass_kernel_spmd

N = 8192          # output rows of W
M = 8192          # contraction dim (input features)
NC = 8            # cores
NPC = N // NC     # 1024 output rows per core
MT = M // 128     # 64 contraction subtiles of 128

F8 = mybir.dt.float8e4
U32 = mybir.dt.uint32
F32 = mybir.dt.float32

WSCALE = np.float32(64.0)  # host pre-scale of W into fp8e4 normal range

# DMA tile schedule, in 128-row m-subtiles per transfer.  1 MiB steady
# tiles keep the SDMA engines saturated (8 KB packets, per-transfer receipt
# stalls amortized and hidden by the other ring); small tiles at both ends
# cut the latency to the first matmul and the drain-out tail.  The phase-
# split matmul order hides the per-tile |W| AND latency.  Must sum to MT.
TILE_SCHED = [4] * 16
assert sum(TILE_SCHED) == MT

N_WARM = 8  # cold-clock PE warmup matmuls issued during the DMA preamble

_nc_cache = {}


def _build():
    nc = bass.Bass()
    # host pre-tiles W^T so each [128, A*NPC] DMA tile is one contiguous
    # block: tile t partition p holds rows {m0_t + a*128 + p} of W^T[:, core]
    wt = nc.dram_tensor("wt", [M * NPC], F8, kind="ExternalInput")
    vecs = nc.dram_tensor("vecs", [128, 2 * MT], F8, kind="ExternalInput")
    # raw PSUM image; host reads rows 0/32/64/96
    outm = nc.dram_tensor("outm", [97, 512], F32, kind="ExternalOutput")

    with TileContext(nc) as tc:
        with (
            tc.tile_pool(name="wpool", bufs=9) as wpool,
            tc.tile_pool(name="wapool", bufs=9) as wapool,
            tc.tile_pool(name="const", bufs=1) as cpool,
            tc.tile_pool(name="psum", bufs=1, space="PSUM") as ppool,
            tc.tile_pool(name="epil", bufs=1) as epool,
        ):
            # vecs rides the GPSIMD SWDGE queue so both HWDGE rings start on
            # W tiles immediately; it lands well before the first real matmul
            vecs_sb = cpool.tile([128, 2 * MT], F8, tag="vecs")
            nc.gpsimd.dma_start(out=vecs_sb[:], in_=vecs[:])

            # PE warmup: dep-free matmuls on memset scratch keep the PE busy
            # through the HAM SHORT window while the first W tile loads, so
            # real matmuls run at 2.4 GHz from the start.
            scratch = cpool.tile([128, 512], F8, tag="scratch")
            nc.gpsimd.memset(scratch[:], 0.0)
            warm_ps = ppool.tile([2, 512], F32, tag="warm", name="warm_ps")
            for _ in range(N_WARM):
                nc.tensor.matmul(
                    warm_ps[:],
                    scratch[:, 0:2],
                    scratch[:, 0:512],
                    start=True,
                    stop=True,
                )

            # accumulators: row 0 = W@s cols 0-511, row 32 = W@s cols
            # 512-1023, row 64 = |W|@d cols 0-511, row 96 = |W|@d cols
            # 512-1023 (output partition == PE column-group offset)
            ps = ppool.tile([97, 512], F32, tag="ps", name="ps")

            mt = 0
            ofs = 0
            # strict alternation of equal-size tiles across the two HWDGE
            # rings (SP / ACT): balanced rings bound the arrival skew to one
            # tile, so the in-order PE queue never waits long for a lagging
            # ring while the other ring's data sits in SBUF.
            for t, A in enumerate(TILE_SCHED):
                w = wpool.tile([128, A * NPC], F8, tag="w", name="w")
                dma_eng = nc.sync if t % 2 == 0 else nc.scalar
                dma_eng.dma_start(
                    out=w[:],
                    in_=wt[ofs : ofs + 128 * A * NPC].rearrange(
                        "(p f) -> p f", p=128
                    ),
                )
                ofs += 128 * A * NPC
                # |W| tile: fp8e4 is sign-magnitude, so clearing the top bit
                # of every byte is elementwise abs; one u32 AND handles 4
                # fp8 lanes -> 2 elem/cycle/partition on the DVE.
                wa = wapool.tile([128, A * NPC // 4], U32, tag="wa", name="wa")
                nc.vector.tensor_scalar(
                    out=wa[:],
                    in0=w[:].bitcast(U32),
                    scalar1=0x7F7F7F7F,
                    scalar2=None,
                    op0=mybir.AluOpType.bitwise_and,
                )
                # emit all W matmuls of the tile first (they only depend on
                # the DMA), then all |W| matmuls (which wait on the AND): the
                # in-order PE queue then never stalls real W work behind the
                # DVE, and adjacent tiles' W/|W| phases overlap so all four
                # col-group streams stay busy.
                for a in range(A):
                    ma = mt + a
                    sv = vecs_sb[:, 2 * ma : 2 * ma + 1]
                    lo = a * NPC
                    st = ma == 0
                    sp = ma == MT - 1
                    nc.tensor.matmul(
                        ps[0:1, :],
                        sv,
                        w[:, lo : lo + 512],
                        start=st,
                        stop=sp,
                        tile_position=(0, 0),
                        skip_group_check=True,
                    )
                    nc.tensor.matmul(
                        ps[32:33, :],
                        sv,
                        w[:, lo + 512 : lo + 1024],
                        start=st,
                        stop=sp,
                        tile_position=(0, 32),
                        skip_group_check=True,
                    )
                for a in range(A):
                    ma = mt + a
                    dv = vecs_sb[:, 2 * ma + 1 : 2 * ma + 2]
                    q = a * NPC // 4
                    st = ma == 0
                    sp = ma == MT - 1
                    nc.tensor.matmul(
                        ps[64:65, :],
                        dv,
                        wa[:, q : q + 128].bitcast(F8),
                        start=st,
                        stop=sp,
                        tile_position=(0, 64),
                        skip_group_check=True,
                    )
                    nc.tensor.matmul(
                        ps[96:97, :],
                        dv,
                        wa[:, q + 128 : q + 256].bitcast(F8),
                        start=st,
                        stop=sp,
                        tile_position=(0, 96),
                        skip_group_check=True,
                    )
                mt += A
                # dep-free filler matmuls on the early tile boundaries keep
                # the PE HAM activity window non-idle through the ramp; in
                # the steady state the supply gaps are ~2 us < the 3.4 us
                # HAM window, so no fillers are needed there.
                if t < 4:
                    for _ in range(2):
                        nc.tensor.matmul(
                            warm_ps[:, 0:256],
                            scratch[:, 0:2],
                            scratch[:, 0:256],
                            start=True,
                            stop=True,
                        )
                elif t < len(TILE_SCHED) - 1:
                    nc.tensor.matmul(
                        warm_ps[:, 0:64],
                        scratch[:, 0:2],
                        scratch[:, 0:64],
                        start=True,
                        stop=True,
                    )

            # evacuate PSUM through SBUF and out to HBM.  Four single-row
            # transfers: a multi-partition SBUF->DRAM transfer serializes all
            # its per-partition descriptors onto ONE SDMA engine (~22 GB/s),
            # so a [97,512] store costs ~9 us while 4 tiny rows cost ~1.5 us.
            om = epool.tile([97, 512], F32, tag="om")
            nc.vector.tensor_copy(om[:], ps[:])
            nc.sync.dma_start(out=outm[0:1, :], in_=om[0:1, :])
            nc.scalar.dma_start(out=outm[32:33, :], in_=om[32:33, :])
            nc.sync.dma_start(out=outm[64:65, :], in_=om[64:65, :])
            nc.scalar.dma_start(out=outm[96:97, :], in_=om[96:97, :])
    return nc


def _legalize_sync_waits(nc):
    """The walrus codegen in this toolchain accepts at most ONE sync-wait per
    instruction ("Too many sync wait commands").  Tile freely attaches
    several.  Hoist all but the last wait of each offending instruction onto
    same-engine NOPs spliced immediately before it — same-queue waits execute
    in order, so semantics are identical."""
    nop_map = {}
    all_nops = set()
    for f in nc.m.functions:
        for b in f.blocks:
            for inst in list(b.instructions):
                si = inst.sync_info
                if not (si and si.on_wait and len(si.on_wait) > 1):
                    continue
                waits = list(si.on_wait)
                nops = []
                for w in waits[:-1]:
                    # engine.nop() appends to the current (last) bb; the
                    # splice below removes it from wherever it landed and
                    # re-inserts it right before its target instruction.
                    nop = nc.engines[inst.engine].nop()
                    nop.ins.sync_info = mybir.SyncInfo(on_wait=[w], on_update=[])
                    nops.append(nop.ins)
                    all_nops.add(nop.ins.name)
                inst.sync_info = mybir.SyncInfo(
                    on_wait=[waits[-1]], on_update=list(si.on_update or [])
                )
                nop_map[inst.name] = nops
    if not nop_map:
        return
    for f in nc.m.functions:
        for b in f.blocks:
            insts = b.instructions
            new_list = []
            for inst in insts:
                if inst.name in all_nops:
                    continue
                for nop in nop_map.get(inst.name, ()):
                    new_list.append(nop)
                new_list.append(inst)
            insts[:] = new_list


def get_nc():
    if "fp8" not in _nc_cache:
        nc = _build()
        _legalize_sync_waits(nc)
        _nc_cache["fp8"] = nc
    return _nc_cache["fp8"]


def host_prep(bounds, weight, bias, in_lower, in_upper):
    f8 = np.dtype(mybir.dt.np(F8))
    f32 = np.float32
    weight = np.asarray(weight, f32)
    in_lower = np.asarray(in_lower, f32)
    in_upper = np.asarray(in_upper, f32)

    s = (in_lower + in_upper).astype(f32)
    d = (in_lower - in_upper).astype(f32)
    # per m-subtile stationary columns: [s, d]
    mvecs = np.stack([s, d], axis=1).astype(f8)
    vecs = np.ascontiguousarray(
        mvecs.reshape(MT, 128, 2).transpose(1, 0, 2).reshape(128, 2 * MT)
    )

    WT = np.ascontiguousarray((weight.T * WSCALE).astype(f8))  # [M, N]
    in_maps = []
    for c in range(NC):
        sl = slice(c * NPC, (c + 1) * NPC)
        Wc = WT[:, sl]
        blocks = []
        m0 = 0
        for A in TILE_SCHED:
            blocks.append(
                Wc[m0 : m0 + A * 128]
                .reshape(A, 128, NPC)
                .transpose(1, 0, 2)
                .reshape(-1)
            )
            m0 += A * 128
        wt_flat = np.ascontiguousarray(np.concatenate(blocks))
        in_maps.append({"wt": wt_flat, "vecs": vecs})
    return in_maps


def assemble(results, bounds, bias):
    """Host epilogue: combine the raw matvecs with the O(N) DeepPoly
    coefficient math, exactly mirroring the reference formulas in fp32."""
    f32 = np.float32
    bounds = np.asarray(bounds, f32)
    bias = np.asarray(bias, f32)
    l, u = bounds[0], bounds[1]
    ind2 = l >= 0
    ind3 = (u > 0) & (l < 0)
    one, zero = f32(1.0), f32(0.0)
    diff = np.where(ind3, u - l, one).astype(f32)
    lmbda = np.where(ind2, one, np.where(ind3, u / diff, zero)).astype(f32)
    beta = np.where(ind2, one, zero).astype(f32)
    mu = np.where(ind3, -l * u / diff, zero).astype(f32)
    lb0 = np.where(ind2, l, zero).astype(f32)
    ub0 = np.where(ind2, u, np.where(ind3, u, zero)).astype(f32)

    a = np.empty(N, f32)
    b = np.empty(N, f32)
    inv = f32(1.0) / (f32(2.0) * WSCALE)
    for c, r in enumerate(results):
        sl = slice(c * NPC, (c + 1) * NPC)
        om = np.asarray(r["outm"], f32)  # raw [97, 512] PSUM image
        ws = np.concatenate([om[0], om[32]])   # W@s, scaled by WSCALE
        ad = np.concatenate([om[64], om[96]])  # |W|@d, scaled by WSCALE
        a[sl] = (ws + ad) * inv
        b[sl] = (ws - ad) * inv

    new_l = (beta * (a + bias)).astype(f32)
    new_u = (lmbda * (b + bias) + mu).astype(f32)
    lb = np.maximum(lb0, new_l)
    ub = np.minimum(ub0, new_u)
    return np.stack([lb, ub]).astype(f32)


def kernel(bounds, weight, bias, in_lower, in_upper):
    nc = get_nc()
    in_maps = host_prep(bounds, weight, bias, in_lower, in_upper)
    res = run_bass_kernel_spmd(nc, in_maps, list(range(NC)))
    return assemble(res.results, bounds, bias)
